# revision 5
# baseline (speedup 1.0000x reference)
"""Trainium2 Bass kernel for the two-branch sparse-attention fusion module.

Math (per batch b, tokens T = rgb/evt as (d=256, N=4096) d-major):
    s      = sum_n T[:, n]                           (256,)
    value[n] = T[:,n].v + c, v = (Wq^T Wk) s + N Wq^T bk, c = (Wk^T bq).s + N bq.bk
    w      = sigmoid((value_rgb - value_evt)/sqrt(d))
    out    = evt + w * (rgb - evt)

Dataflow (fp16 compute, fp32 DRAM I/O + PSUM accumulation):
    SP HWDGE   : packed weight/bias load (f32, 2 contiguous DMAs)
    gpsimd     : casting token loads f32->fp16, partition_broadcast of the
                 sigmoid row to 128 partitions, casting stores fp16->f32
    ScalarE    : streaming row-sum partials of A, sigmoid (psv->fp16 row)
    PE         : weight-product precompute, per-batch head matvecs, fp16
                 value matmuls (no hi/lo split; rel err ~1.3e-3 << 2e-2)
    DVE        : M = A-D with rowsum accum, blend M*=wb, A=M+D (fp16 2x)

Chunk-granular software pipeline. Each engine's program order is pinned
with explicit dependency chains (seq) in measured data-arrival order so
the tile scheduler cannot head-of-line block a late-data op ahead of
ready work. The last batch-1 blocks are split finer to shorten the tail.
Sharded data-parallel over batch: 8 cores x 2 batches, weights replicated.
"""

import numpy as np
from contextlib import ExitStack

import concourse.bass as bass
import concourse.tile as tile
from concourse import bacc, mybir
from concourse.bass import _add_dep_helper
from concourse.bass_utils import run_bass_kernel_spmd

F32 = mybir.dt.float32
FP16 = mybir.dt.float16

BS, DIM, HH, WW = 16, 256, 64, 64
N = HH * WW                 # 4096 tokens
NCORES = 8
BPC = BS // NCORES          # batches per core
PH = DIM // 128             # partition halves of the d dim
CH = 512                    # value-chunk (one PSUM bank of f32)
NCH = N // CH               # 8
LB = 2048                   # load block columns (1 MiB DRAM-side)
NLB = N // LB               # 2
PB = 1024                   # tail blend piece columns
INV_SQRT_D = 1.0 / 16.0


def build_nc() -> bass.Bass:
    nc = bacc.Bacc()

    rgb = nc.declare_dram_parameter("rgb", [BPC, PH, 128, N], F32, isOutput=False)
    evt = nc.declare_dram_parameter("evt", [BPC, PH, 128, N], F32, isOutput=False)
    # host-side packed weights/biases: one contiguous f32 row per partition
    # so the SP HWDGE load is a single clean 128-descriptor DMA each
    wpack = nc.declare_dram_parameter("wpack", [128, 4 * PH * DIM], F32, isOutput=False)
    bpack = nc.declare_dram_parameter("bpack", [128, 4 * PH], F32, isOutput=False)
    out = nc.declare_dram_parameter("out", [BPC, PH, 128, N], F32, isOutput=True)

    with tile.TileContext(nc) as tc:
        _body(tc, rgb, evt, wpack, bpack, out)
    nc.finalize()
    return nc


def _body(tc, rgb, evt, wpack, bpack, out):
    nc = tc.nc
    ACT = mybir.ActivationFunctionType

    chains = {}

    def seq(key, inst):
        # pin program order on one engine: inst runs after the previous
        # chained inst, making emission order authoritative
        prev = chains.get(key)
        if prev is not None:
            _add_dep_helper(inst.ins, prev.ins, sync=True, reason=f"order-{key}")
        chains[key] = inst
        return inst

    with ExitStack() as ctx:
        consts = ctx.enter_context(tc.tile_pool(name="consts", bufs=1))
        data = ctx.enter_context(tc.tile_pool(name="data", bufs=2))
        mpool = ctx.enter_context(tc.tile_pool(name="mpool", bufs=2))
        wbp = ctx.enter_context(tc.tile_pool(name="wbp", bufs=2))
        small = ctx.enter_context(tc.tile_pool(name="small", bufs=2))
        wrp = ctx.enter_context(tc.tile_pool(name="wrp", bufs=4))
        ps_val = ctx.enter_context(tc.tile_pool(name="ps_val", bufs=4, space="PSUM"))
        ps_pre = ctx.enter_context(tc.tile_pool(name="ps_pre", bufs=2, space="PSUM"))
        ps_head = ctx.enter_context(tc.tile_pool(name="ps_head", bufs=2, space="PSUM"))

        one_one = consts.tile([1, 1], FP16, tag="one_one")
        seq("dve", nc.vector.memset(one_one, 1.0))
        garbage = consts.tile([128, 1], F32, tag="garbage")
        sig_warm = consts.tile([1, 1], F32, tag="sig_warm")

        # ---- weight loads (SP HWDGE, contiguous) ----------------------
        W, B = {}, {}
        wt = consts.tile([128, 4 * PH * DIM], F32, tag="wpack")
        nc.sync.dma_start(out=wt, in_=wpack[:, :])
        bt = consts.tile([128, 4 * PH], F32, tag="bpack")
        nc.sync.dma_start(out=bt, in_=bpack[:, :])
        for wi, nm in enumerate(("Wq_a", "Wk_a", "Wq_d", "Wk_d")):
            for h in range(PH):
                base = (wi * PH + h) * DIM
                W[(nm, h)] = wt[:, base : base + DIM]
        for bi, nm in enumerate(("bq_a", "bk_a", "bq_d", "bk_d")):
            for h in range(PH):
                B[(nm, h)] = bt[:, bi * PH + h : bi * PH + h + 1]

        # ---- token loads ----------------------------------------------
        # b0: 8 x 1MiB blocks. b1: same but the final (h1, blk1) pair is
        # split into 1024-col halves so the tail sub/red start sooner.
        st = [dict() for _ in range(BPC)]
        for b in range(BPC):
            A, Dv = {}, {}
            for h in range(PH):
                A[h] = data.tile([128, N], FP16, tag=f"A{h}", name=f"A{h}_{b}")
                Dv[h] = data.tile([128, N], FP16, tag=f"D{h}", name=f"D{h}_{b}")
            st[b].update(A=A, Dv=Dv)

        def emit_load(b, h, c0, c1):
            sl = slice(c0, c1)
            seq("gp", nc.gpsimd.dma_start(out=st[b]["A"][h][:, sl], in_=rgb[b, h][:, sl]))
            seq("gp", nc.gpsimd.dma_start(out=st[b]["Dv"][h][:, sl], in_=evt[b, h][:, sl]))

        for blk in range(NLB):
            for h in range(PH):
                emit_load(0, h, blk * LB, (blk + 1) * LB)
        emit_load(1, 0, 0, LB)
        emit_load(1, 1, 0, LB)
        emit_load(1, 0, LB, 2 * LB)
        emit_load(1, 1, LB, LB + PB)
        emit_load(1, 1, LB + PB, 2 * LB)

        # ---- precompute (PE on f32 weights; DVE casts) ----------------
        PT, U, R = {}, {}, {}
        for br, wq, wk, sign in (("a", "Wq_a", "Wk_a", 1.0), ("d", "Wq_d", "Wk_d", -1.0)):
            for jh in range(PH):
                ps = ps_pre.tile([128, DIM], F32, tag="ps_pre", name=f"psPT{br}{jh}")
                for oh in range(PH):
                    seq("pe", nc.tensor.matmul(
                        ps,
                        lhsT=W[(wk, oh)][:, jh * 128 : (jh + 1) * 128],
                        rhs=W[(wq, oh)],
                        start=(oh == 0), stop=(oh == PH - 1),
                    ))
                t = consts.tile([128, DIM], FP16, tag=f"PT{br}{jh}", name=f"PT{br}{jh}")
                seq("dve", nc.vector.tensor_scalar_mul(out=t, in0=ps, scalar1=sign))
                PT[(br, jh)] = t
            ps = ps_pre.tile([128, 2 * PH], F32, tag="ps_pre", name=f"psUR{br}")
            for ih in range(PH):
                for oh in range(PH):
                    seq("pe", nc.tensor.matmul(
                        ps[:, ih : ih + 1],
                        lhsT=W[(wq, oh)][:, ih * 128 : (ih + 1) * 128],
                        rhs=B[("bk_" + br, oh)],
                        start=(oh == 0), stop=(oh == PH - 1),
                    ))
            for jh in range(PH):
                for oh in range(PH):
                    seq("pe", nc.tensor.matmul(
                        ps[:, PH + jh : PH + jh + 1],
                        lhsT=W[(wk, oh)][:, jh * 128 : (jh + 1) * 128],
                        rhs=B[("bq_" + br, oh)],
                        start=(oh == 0), stop=(oh == PH - 1),
                    ))
            tU = consts.tile([128, PH], F32, tag=f"U{br}", name=f"U{br}")
            seq("dve", nc.vector.tensor_scalar_mul(out=tU, in0=ps[:, 0:PH], scalar1=float(sign * N)))
            tR = consts.tile([128, PH], FP16, tag=f"R{br}", name=f"R{br}")
            seq("dve", nc.vector.tensor_scalar_mul(out=tR, in0=ps[:, PH : 2 * PH], scalar1=sign))
            U[br], R[br] = tU, tR

        # ---- stage 1 ops ----------------------------------------------
        # partial-sum slots per batch: list of (h, c0, c1)
        SLOTS = [
            [(0, 0, LB), (1, 0, LB), (0, LB, 2 * LB), (1, LB, 2 * LB)],
            [(0, 0, LB), (1, 0, LB), (0, LB, 2 * LB), (1, LB, LB + PB), (1, LB + PB, 2 * LB)],
        ]
        for b in range(BPC):
            ns = len(SLOTS[b])
            st[b]["sa4"] = small.tile([128, ns], F32, tag="sa4", name=f"sa4_{b}")
            st[b]["sm4"] = small.tile([128, ns], F32, tag="sm4", name=f"sm4_{b}")
            st[b]["sa16"] = small.tile([128, ns], FP16, tag="sa16", name=f"sa16_{b}")
            st[b]["sd16"] = small.tile([128, ns], FP16, tag="sd16", name=f"sd16_{b}")
            M = {}
            for h in range(PH):
                M[h] = mpool.tile([128, N], FP16, tag=f"M{h}", name=f"M{h}_{b}")
            st[b]["M"] = M

        def red(b, i):
            h, c0, c1 = SLOTS[b][i]
            return seq("act", nc.scalar.activation(
                out=garbage.broadcast_to([128, c1 - c0]),
                in_=st[b]["A"][h][:, c0:c1],
                func=ACT.Copy,
                accum_out=st[b]["sa4"][:, i : i + 1],
            ))

        def sub(b, i):
            h, c0, c1 = SLOTS[b][i]
            return seq("dve", nc.vector.scalar_tensor_tensor(
                out=st[b]["M"][h][:, c0:c1],
                in0=st[b]["A"][h][:, c0:c1],
                scalar=1.0,
                in1=st[b]["Dv"][h][:, c0:c1],
                op0=mybir.AluOpType.mult,
                op1=mybir.AluOpType.subtract,
                accum_out=st[b]["sm4"][:, i : i + 1],
            ))

        def derive(b):
            with nc.allow_low_precision(reason="tiny fp16 partials"):
                seq("dve", nc.vector.tensor_scalar_mul(
                    out=st[b]["sa16"], in0=st[b]["sa4"], scalar1=1.0))
                seq("dve", nc.vector.tensor_sub(
                    out=st[b]["sd16"], in0=st[b]["sa4"], in1=st[b]["sm4"]))

        # batch-independent bias-dot part of c_diff: N*(bq_a.bk_a - bq_d.bk_d)
        ps = ps_pre.tile([1, 1], F32, tag="ps_pre", name="psCb")
        k = 0
        for bq, bk, sgn in (("bq_a", "bk_a", 1), ("bq_d", "bk_d", -1)):
            for oh in range(PH):
                t = consts.tile([128, 1], F32, tag=f"bkN{bk}{oh}", name=f"bkN{bk}{oh}")
                seq("dve", nc.vector.tensor_scalar_mul(
                    out=t, in0=B[(bk, oh)], scalar1=float(sgn * N)))
                seq("pe", nc.tensor.matmul(ps, lhsT=B[(bq, oh)], rhs=t,
                                           start=(k == 0), stop=(k == 3)))
                k += 1
        c_bias = consts.tile([1, 1], FP16, tag="c_bias")
        seq("dve", nc.vector.tensor_scalar_mul(out=c_bias, in0=ps, scalar1=1.0))

        # ---- head -----------------------------------------------------
        def head_pe(b):
            sa16, sd16 = st[b]["sa16"], st[b]["sd16"]
            S4 = {"a": sa16, "d": sd16}
            ps_c = ps_head.tile([1, 1], F32, tag="ps_h", name=f"psc_{b}")
            terms = [
                (S4[br][:, i : i + 1], R[br][:, SLOTS[b][i][0] : SLOTS[b][i][0] + 1])
                for br in ("a", "d")
                for i in range(len(SLOTS[b]))
            ]
            for i, (l, r) in enumerate(terms):
                seq("pe", nc.tensor.matmul(ps_c, lhsT=l, rhs=r, start=(i == 0), stop=False))
            seq("pe", nc.tensor.matmul(ps_c, lhsT=c_bias, rhs=one_one, start=False, stop=True))
            psv = {}
            for br in ("a", "d"):
                ps = ps_head.tile([128, PH], F32, tag="ps_h", name=f"psv{br}_{b}")
                for ih in range(PH):
                    nslots = len(SLOTS[b])
                    for i in range(nslots):
                        jh = SLOTS[b][i][0]
                        seq("pe", nc.tensor.matmul(
                            ps[:, ih : ih + 1],
                            lhsT=PT[(br, jh)][:, ih * 128 : (ih + 1) * 128],
                            rhs=S4[br][:, i : i + 1],
                            start=(i == 0), stop=(i == nslots - 1),
                        ))
                psv[br] = ps
            st[b]["ps_c"], st[b]["ps_v"] = ps_c, psv

        def head_c16(b):
            c16 = small.tile([1, 1], F32, tag="c16", name=f"c16_{b}")
            seq("act", nc.scalar.mul(out=c16, in_=st[b]["ps_c"], mul=INV_SQRT_D))
            st[b]["c16"] = c16

        def head_v(b):
            VH = {}
            for br in ("a", "d"):
                v = small.tile([128, PH], F32, tag=f"v{br}", name=f"v{br}_{b}")
                seq("dve", nc.vector.tensor_add(out=v, in0=st[b]["ps_v"][br], in1=U[br]))
                vh = small.tile([128, PH], FP16, tag=f"vh{br}", name=f"vh{br}_{b}")
                with nc.allow_low_precision(reason="fp16 matvec vector"):
                    seq("dve", nc.vector.tensor_scalar_mul(out=vh, in0=v, scalar1=1.0))
                VH[br] = vh
            st[b]["VH"] = VH

        # ---- stage 2: per 512-chunk value -> sigmoid -> broadcast -----
        for b in range(BPC):
            st[b]["wb_sb"] = wbp.tile([128, N], FP16, tag="wb_sb", name=f"wb_sb_{b}")
            st[b]["wr"] = {}

        def chunk_pe(b, ich):
            VH, A, Dv = st[b]["VH"], st[b]["A"], st[b]["Dv"]
            sl = slice(ich * CH, (ich + 1) * CH)
            psv = ps_val.tile([1, CH], F32, tag="psv", name=f"psval{ich}_{b}")
            mms = [
                (VH["a"][:, 0:1], A[0]), (VH["a"][:, 1:2], A[1]),
                (VH["d"][:, 0:1], Dv[0]), (VH["d"][:, 1:2], Dv[1]),
            ]
            for i, (v, t) in enumerate(mms):
                seq("pe", nc.tensor.matmul(psv, lhsT=v, rhs=t[:, sl],
                                           start=(i == 0), stop=(i == len(mms) - 1)))
            st[b]["wr"][ich] = psv

        def chunk_sig(b, ich):
            wrow = wrp.tile([1, CH], FP16, tag="wr", name=f"wrow{ich}_{b}")
            seq("act", nc.scalar.activation(
                out=wrow, in_=st[b]["wr"][ich],
                func=ACT.Sigmoid, bias=st[b]["c16"], scale=INV_SQRT_D,
            ))
            st[b]["wr"][ich] = wrow

        def chunk_bc(b, ich):
            seq("gp", nc.gpsimd.partition_broadcast(
                st[b]["wb_sb"][:, ich * CH : (ich + 1) * CH], st[b]["wr"][ich]))

        # ---- blend + store --------------------------------------------
        def mul(b, h, c0, c1):
            M, wb = st[b]["M"], st[b]["wb_sb"]
            seq("dve", nc.vector.tensor_mul(
                out=M[h][:, c0:c1], in0=M[h][:, c0:c1], in1=wb[:, c0:c1]))

        def add(b, h, c0, c1):
            A, Dv, M = st[b]["A"], st[b]["Dv"], st[b]["M"]
            seq("dve", nc.vector.tensor_add(
                out=A[h][:, c0:c1], in0=M[h][:, c0:c1], in1=Dv[h][:, c0:c1]))

        def store(b, h, c0, c1):
            seq("gp", nc.gpsimd.dma_start(
                out=out[b, h][:, c0:c1], in_=st[b]["A"][h][:, c0:c1]))

        # ---- emission schedule ----------------------------------------
        # scalar: warm the sigmoid table set during the idle load phase
        seq("act", nc.scalar.activation(out=sig_warm, in_=one_one,
                                        func=ACT.Sigmoid, bias=0.0, scale=1.0))
        for i in range(4):
            red(0, i)
        sub(0, 0)
        sub(0, 1)
        sub(0, 2)
        sub(0, 3)
        derive(0)
        head_pe(0)
        head_c16(0)
        head_v(0)

        red(1, 0)
        chunk_pe(0, 0); chunk_sig(0, 0); chunk_bc(0, 0)
        chunk_pe(0, 1); chunk_sig(0, 1); chunk_bc(0, 1)
        red(1, 1)
        sub(1, 0)
        sub(1, 1)
        for ich in range(2, 8):
            chunk_pe(0, ich); chunk_sig(0, ich); chunk_bc(0, ich)
        # blend0 col-block 0 while b1 tail loads land
        mul(0, 0, 0, LB); add(0, 0, 0, LB)
        mul(0, 1, 0, LB); add(0, 1, 0, LB)
        store(0, 0, 0, LB); store(0, 1, 0, LB)
        red(1, 2)
        red(1, 3)
        red(1, 4)
        sub(1, 2)
        sub(1, 3)
        sub(1, 4)
        derive(1)
        head_pe(1)
        head_c16(1)
        head_v(1)
        # blend0 col-block 1 on DVE while PE runs b1 values
        mul(0, 0, LB, N); add(0, 0, LB, N)
        mul(0, 1, LB, N); add(0, 1, LB, N)
        for ich in range(3):
            chunk_pe(1, ich); chunk_sig(1, ich); chunk_bc(1, ich)
        # b0 col-block-1 stores fill the DMA gap while b1 chunks trickle
        store(0, 0, LB, N); store(0, 1, LB, N)
        for ich in range(3, 8):
            chunk_pe(1, ich); chunk_sig(1, ich); chunk_bc(1, ich)
        # blend1 in 1024-col pieces trickling into stores
        for p in range(4):
            c0, c1 = p * PB, (p + 1) * PB
            mul(1, 0, c0, c1); add(1, 0, c0, c1)
            mul(1, 1, c0, c1); add(1, 1, c0, c1)
            store(1, 0, c0, c1); store(1, 1, c0, c1)


_NC_CACHE = None


def _get_nc():
    global _NC_CACHE
    if _NC_CACHE is None:
        _NC_CACHE = build_nc()
    return _NC_CACHE


def _make_in_maps(inputs):
    rgb = np.ascontiguousarray(np.asarray(inputs["rgb"], dtype=np.float32)).reshape(
        BS, PH, 128, N
    )
    evt = np.ascontiguousarray(np.asarray(inputs["evt"], dtype=np.float32)).reshape(
        BS, PH, 128, N
    )
    wpack = np.stack(
        [
            np.asarray(inputs[nm], dtype=np.float32).reshape(PH, 128, DIM).transpose(1, 0, 2)
            for nm in ("Wq_a", "Wk_a", "Wq_d", "Wk_d")
        ],
        axis=1,
    ).reshape(128, 4 * PH * DIM)
    bpack = np.stack(
        [
            np.asarray(inputs[nm], dtype=np.float32).reshape(PH, 128).T
            for nm in ("bq_a", "bk_a", "bq_d", "bk_d")
        ],
        axis=1,
    ).reshape(128, 4 * PH)
    base = {"wpack": np.ascontiguousarray(wpack), "bpack": np.ascontiguousarray(bpack)}
    in_maps = []
    for c in range(NCORES):
        m = dict(base)
        m["rgb"] = np.ascontiguousarray(rgb[c * BPC : (c + 1) * BPC])
        m["evt"] = np.ascontiguousarray(evt[c * BPC : (c + 1) * BPC])
        in_maps.append(m)
    return in_maps


def run(inputs, trace=False):
    nc = _get_nc()
    in_maps = _make_in_maps(inputs)
    res = run_bass_kernel_spmd(nc, in_maps, core_ids=list(range(NCORES)), trace=trace)
    outs = [
        np.asarray(res.results[i]["out"]).reshape(BPC, DIM, HH, WW)
        for i in range(NCORES)
    ]
    full = np.concatenate(outs, axis=0)
    return full, res


def kernel(**inputs) -> np.ndarray:
    full, _ = run(inputs, trace=False)
    return full


# revision 6
# speedup vs baseline: 1.9128x; 1.9128x over previous
"""Trainium2 Bass kernel for the two-branch sparse-attention fusion module.

Math (per batch b, tokens T = rgb/evt as (d=256, N=4096) d-major):
    s      = sum_n T[:, n]                           (256,)
    value[n] = T[:,n].v + c, v = (Wq^T Wk) s + N Wq^T bk, c = (Wk^T bq).s + N bq.bk
    w      = sigmoid((value_rgb - value_evt)/sqrt(d))
    out    = evt + w * (rgb - evt)

Dataflow (fp16 compute, fp32 DRAM I/O + PSUM accumulation):
    SP HWDGE   : packed weight/bias load (f32, 2 contiguous DMAs)
    gpsimd     : casting token loads f32->fp16, partition_broadcast of the
                 sigmoid row to 128 partitions, casting stores fp16->f32
    ScalarE    : streaming row-sum partials of A, sigmoid (psv->fp16 row)
    PE         : weight-product precompute, per-batch head matvecs, fp16
                 value matmuls (no hi/lo split; rel err ~1.3e-3 << 2e-2)
    DVE        : M = A-D with rowsum accum, blend M*=wb, A=M+D (fp16 2x)

Chunk-granular software pipeline. Each engine's program order is pinned
with explicit dependency chains (seq) in measured data-arrival order so
the tile scheduler cannot head-of-line block a late-data op ahead of
ready work. The last batch-1 blocks are split finer to shorten the tail.
Sharded data-parallel over batch: 8 cores x 2 batches, weights replicated.
"""

import numpy as np
from contextlib import ExitStack

import concourse.bass as bass
import concourse.tile as tile
from concourse import bacc, mybir
from concourse.bass import _add_dep_helper
from concourse.bass_utils import run_bass_kernel_spmd

F32 = mybir.dt.float32
FP16 = mybir.dt.float16

BS, DIM, HH, WW = 16, 256, 64, 64
N = HH * WW                 # 4096 tokens
NCORES = 8
BPC = BS // NCORES          # batches per core
PH = DIM // 128             # partition halves of the d dim
CH = 512                    # value-chunk (one PSUM bank of f32)
NCH = N // CH               # 8
LB = 2048                   # load block columns (1 MiB DRAM-side)
NLB = N // LB               # 2
PB = 1024                   # tail blend piece columns
INV_SQRT_D = 1.0 / 16.0


def build_nc() -> bass.Bass:
    nc = bacc.Bacc()

    rgb = nc.declare_dram_parameter("rgb", [BPC, PH, 128, N], F32, isOutput=False)
    evt = nc.declare_dram_parameter("evt", [BPC, PH, 128, N], F32, isOutput=False)
    # host-side packed weights/biases: one contiguous f32 row per partition
    # so the SP HWDGE load is a single clean 128-descriptor DMA each
    wpack = nc.declare_dram_parameter("wpack", [128, 4 * PH * DIM], F32, isOutput=False)
    bpack = nc.declare_dram_parameter("bpack", [128, 4 * PH], F32, isOutput=False)
    out = nc.declare_dram_parameter("out", [BPC, PH, 128, N], F32, isOutput=True)

    with tile.TileContext(nc) as tc:
        _body(tc, rgb, evt, wpack, bpack, out)
    nc.finalize()
    return nc


def _body(tc, rgb, evt, wpack, bpack, out):
    nc = tc.nc
    ACT = mybir.ActivationFunctionType

    chains = {}

    def seq(key, inst):
        # pin program order on one engine: inst runs after the previous
        # chained inst, making emission order authoritative
        prev = chains.get(key)
        if prev is not None:
            _add_dep_helper(inst.ins, prev.ins, sync=False, reason=f"order-{key}")
        chains[key] = inst
        return inst

    with ExitStack() as ctx:
        consts = ctx.enter_context(tc.tile_pool(name="consts", bufs=1))
        data = ctx.enter_context(tc.tile_pool(name="data", bufs=2))
        mpool = ctx.enter_context(tc.tile_pool(name="mpool", bufs=2))
        wbp = ctx.enter_context(tc.tile_pool(name="wbp", bufs=2))
        small = ctx.enter_context(tc.tile_pool(name="small", bufs=2))
        wrp = ctx.enter_context(tc.tile_pool(name="wrp", bufs=4))
        ps_val = ctx.enter_context(tc.tile_pool(name="ps_val", bufs=4, space="PSUM"))
        ps_pre = ctx.enter_context(tc.tile_pool(name="ps_pre", bufs=2, space="PSUM"))
        ps_head = ctx.enter_context(tc.tile_pool(name="ps_head", bufs=2, space="PSUM"))

        one_one = consts.tile([1, 1], FP16, tag="one_one")
        seq("dve", nc.vector.memset(one_one, 1.0))
        garbage = consts.tile([128, 1], F32, tag="garbage")
        sig_warm = consts.tile([1, 1], F32, tag="sig_warm")

        # ---- weight loads (SP HWDGE, contiguous) ----------------------
        W, B = {}, {}
        wt = consts.tile([128, 4 * PH * DIM], F32, tag="wpack")
        nc.sync.dma_start(out=wt, in_=wpack[:, :])
        bt = consts.tile([128, 4 * PH], F32, tag="bpack")
        nc.sync.dma_start(out=bt, in_=bpack[:, :])
        for wi, nm in enumerate(("Wq_a", "Wk_a", "Wq_d", "Wk_d")):
            for h in range(PH):
                base = (wi * PH + h) * DIM
                W[(nm, h)] = wt[:, base : base + DIM]
        for bi, nm in enumerate(("bq_a", "bk_a", "bq_d", "bk_d")):
            for h in range(PH):
                B[(nm, h)] = bt[:, bi * PH + h : bi * PH + h + 1]

        # ---- token loads ----------------------------------------------
        # b0: 8 x 1MiB blocks. b1: same but the final (h1, blk1) pair is
        # split into 1024-col halves so the tail sub/red start sooner.
        st = [dict() for _ in range(BPC)]
        for b in range(BPC):
            A, Dv = {}, {}
            for h in range(PH):
                A[h] = data.tile([128, N], FP16, tag=f"A{h}", name=f"A{h}_{b}")
                Dv[h] = data.tile([128, N], FP16, tag=f"D{h}", name=f"D{h}_{b}")
            st[b].update(A=A, Dv=Dv)

        def emit_load(b, h, c0, c1):
            sl = slice(c0, c1)
            seq("gp", nc.gpsimd.dma_start(out=st[b]["A"][h][:, sl], in_=rgb[b, h][:, sl]))
            seq("gp", nc.gpsimd.dma_start(out=st[b]["Dv"][h][:, sl], in_=evt[b, h][:, sl]))

        for blk in range(NLB):
            for h in range(PH):
                emit_load(0, h, blk * LB, (blk + 1) * LB)
        emit_load(1, 0, 0, LB)
        emit_load(1, 1, 0, LB)
        emit_load(1, 0, LB, 2 * LB)
        emit_load(1, 1, LB, LB + PB)
        emit_load(1, 1, LB + PB, 2 * LB)

        # ---- precompute (PE on f32 weights; DVE casts) ----------------
        PT, U, R = {}, {}, {}
        for br, wq, wk, sign in (("a", "Wq_a", "Wk_a", 1.0), ("d", "Wq_d", "Wk_d", -1.0)):
            for jh in range(PH):
                ps = ps_pre.tile([128, DIM], F32, tag="ps_pre", name=f"psPT{br}{jh}")
                for oh in range(PH):
                    seq("pe", nc.tensor.matmul(
                        ps,
                        lhsT=W[(wk, oh)][:, jh * 128 : (jh + 1) * 128],
                        rhs=W[(wq, oh)],
                        start=(oh == 0), stop=(oh == PH - 1),
                    ))
                t = consts.tile([128, DIM], FP16, tag=f"PT{br}{jh}", name=f"PT{br}{jh}")
                seq("dve", nc.vector.tensor_scalar_mul(out=t, in0=ps, scalar1=sign))
                PT[(br, jh)] = t
            ps = ps_pre.tile([128, 2 * PH], F32, tag="ps_pre", name=f"psUR{br}")
            for ih in range(PH):
                for oh in range(PH):
                    seq("pe", nc.tensor.matmul(
                        ps[:, ih : ih + 1],
                        lhsT=W[(wq, oh)][:, ih * 128 : (ih + 1) * 128],
                        rhs=B[("bk_" + br, oh)],
                        start=(oh == 0), stop=(oh == PH - 1),
                    ))
            for jh in range(PH):
                for oh in range(PH):
                    seq("pe", nc.tensor.matmul(
                        ps[:, PH + jh : PH + jh + 1],
                        lhsT=W[(wk, oh)][:, jh * 128 : (jh + 1) * 128],
                        rhs=B[("bq_" + br, oh)],
                        start=(oh == 0), stop=(oh == PH - 1),
                    ))
            tU = consts.tile([128, PH], F32, tag=f"U{br}", name=f"U{br}")
            seq("dve", nc.vector.tensor_scalar_mul(out=tU, in0=ps[:, 0:PH], scalar1=float(sign * N)))
            tR = consts.tile([128, PH], FP16, tag=f"R{br}", name=f"R{br}")
            seq("dve", nc.vector.tensor_scalar_mul(out=tR, in0=ps[:, PH : 2 * PH], scalar1=sign))
            U[br], R[br] = tU, tR

        # ---- stage 1 ops ----------------------------------------------
        # partial-sum slots per batch: list of (h, c0, c1)
        SLOTS = [
            [(0, 0, LB), (1, 0, LB), (0, LB, 2 * LB), (1, LB, 2 * LB)],
            [(0, 0, LB), (1, 0, LB), (0, LB, 2 * LB), (1, LB, LB + PB), (1, LB + PB, 2 * LB)],
        ]
        for b in range(BPC):
            ns = len(SLOTS[b])
            st[b]["sa4"] = small.tile([128, ns], F32, tag="sa4", name=f"sa4_{b}")
            st[b]["sm4"] = small.tile([128, ns], F32, tag="sm4", name=f"sm4_{b}")
            st[b]["sa16"] = small.tile([128, ns], FP16, tag="sa16", name=f"sa16_{b}")
            st[b]["sd16"] = small.tile([128, ns], FP16, tag="sd16", name=f"sd16_{b}")
            M = {}
            for h in range(PH):
                M[h] = mpool.tile([128, N], FP16, tag=f"M{h}", name=f"M{h}_{b}")
            st[b]["M"] = M

        def red(b, i):
            h, c0, c1 = SLOTS[b][i]
            return seq("act", nc.scalar.activation(
                out=garbage.broadcast_to([128, c1 - c0]),
                in_=st[b]["A"][h][:, c0:c1],
                func=ACT.Copy,
                accum_out=st[b]["sa4"][:, i : i + 1],
            ))

        def sub(b, i):
            h, c0, c1 = SLOTS[b][i]
            return seq("dve", nc.vector.scalar_tensor_tensor(
                out=st[b]["M"][h][:, c0:c1],
                in0=st[b]["A"][h][:, c0:c1],
                scalar=1.0,
                in1=st[b]["Dv"][h][:, c0:c1],
                op0=mybir.AluOpType.mult,
                op1=mybir.AluOpType.subtract,
                accum_out=st[b]["sm4"][:, i : i + 1],
            ))

        def derive(b):
            with nc.allow_low_precision(reason="tiny fp16 partials"):
                seq("dve", nc.vector.tensor_scalar_mul(
                    out=st[b]["sa16"], in0=st[b]["sa4"], scalar1=1.0))
                seq("dve", nc.vector.tensor_sub(
                    out=st[b]["sd16"], in0=st[b]["sa4"], in1=st[b]["sm4"]))

        # batch-independent bias-dot part of c_diff: N*(bq_a.bk_a - bq_d.bk_d)
        ps = ps_pre.tile([1, 1], F32, tag="ps_pre", name="psCb")
        k = 0
        for bq, bk, sgn in (("bq_a", "bk_a", 1), ("bq_d", "bk_d", -1)):
            for oh in range(PH):
                t = consts.tile([128, 1], F32, tag=f"bkN{bk}{oh}", name=f"bkN{bk}{oh}")
                seq("dve", nc.vector.tensor_scalar_mul(
                    out=t, in0=B[(bk, oh)], scalar1=float(sgn * N)))
                seq("pe", nc.tensor.matmul(ps, lhsT=B[(bq, oh)], rhs=t,
                                           start=(k == 0), stop=(k == 3)))
                k += 1
        c_bias = consts.tile([1, 1], FP16, tag="c_bias")
        seq("dve", nc.vector.tensor_scalar_mul(out=c_bias, in0=ps, scalar1=1.0))

        # ---- head -----------------------------------------------------
        def head_pe(b):
            sa16, sd16 = st[b]["sa16"], st[b]["sd16"]
            S4 = {"a": sa16, "d": sd16}
            ps_c = ps_head.tile([1, 1], F32, tag="ps_h", name=f"psc_{b}")
            terms = [
                (S4[br][:, i : i + 1], R[br][:, SLOTS[b][i][0] : SLOTS[b][i][0] + 1])
                for br in ("a", "d")
                for i in range(len(SLOTS[b]))
            ]
            for i, (l, r) in enumerate(terms):
                seq("pe", nc.tensor.matmul(ps_c, lhsT=l, rhs=r, start=(i == 0), stop=False))
            seq("pe", nc.tensor.matmul(ps_c, lhsT=c_bias, rhs=one_one, start=False, stop=True))
            psv = {}
            for br in ("a", "d"):
                ps = ps_head.tile([128, PH], F32, tag="ps_h", name=f"psv{br}_{b}")
                for ih in range(PH):
                    nslots = len(SLOTS[b])
                    for i in range(nslots):
                        jh = SLOTS[b][i][0]
                        seq("pe", nc.tensor.matmul(
                            ps[:, ih : ih + 1],
                            lhsT=PT[(br, jh)][:, ih * 128 : (ih + 1) * 128],
                            rhs=S4[br][:, i : i + 1],
                            start=(i == 0), stop=(i == nslots - 1),
                        ))
                psv[br] = ps
            st[b]["ps_c"], st[b]["ps_v"] = ps_c, psv

        def head_c16(b):
            c16 = small.tile([1, 1], F32, tag="c16", name=f"c16_{b}")
            seq("act", nc.scalar.mul(out=c16, in_=st[b]["ps_c"], mul=INV_SQRT_D))
            st[b]["c16"] = c16

        def head_v(b):
            VH = {}
            for br in ("a", "d"):
                v = small.tile([128, PH], F32, tag=f"v{br}", name=f"v{br}_{b}")
                seq("dve", nc.vector.tensor_add(out=v, in0=st[b]["ps_v"][br], in1=U[br]))
                vh = small.tile([128, PH], FP16, tag=f"vh{br}", name=f"vh{br}_{b}")
                with nc.allow_low_precision(reason="fp16 matvec vector"):
                    seq("dve", nc.vector.tensor_scalar_mul(out=vh, in0=v, scalar1=1.0))
                VH[br] = vh
            st[b]["VH"] = VH

        # ---- stage 2: per 512-chunk value -> sigmoid -> broadcast -----
        for b in range(BPC):
            st[b]["wb_sb"] = wbp.tile([128, N], FP16, tag="wb_sb", name=f"wb_sb_{b}")
            st[b]["wr"] = {}

        def chunk_pe(b, ich):
            VH, A, Dv = st[b]["VH"], st[b]["A"], st[b]["Dv"]
            sl = slice(ich * CH, (ich + 1) * CH)
            psv = ps_val.tile([1, CH], F32, tag="psv", name=f"psval{ich}_{b}")
            mms = [
                (VH["a"][:, 0:1], A[0]), (VH["a"][:, 1:2], A[1]),
                (VH["d"][:, 0:1], Dv[0]), (VH["d"][:, 1:2], Dv[1]),
            ]
            for i, (v, t) in enumerate(mms):
                seq("pe", nc.tensor.matmul(psv, lhsT=v, rhs=t[:, sl],
                                           start=(i == 0), stop=(i == len(mms) - 1)))
            st[b]["wr"][ich] = psv

        def chunk_sig(b, ich):
            wrow = wrp.tile([1, CH], FP16, tag="wr", name=f"wrow{ich}_{b}")
            seq("act", nc.scalar.activation(
                out=wrow, in_=st[b]["wr"][ich],
                func=ACT.Sigmoid, bias=st[b]["c16"], scale=INV_SQRT_D,
            ))
            st[b]["wr"][ich] = wrow

        def chunk_bc(b, ich):
            seq("gp", nc.gpsimd.partition_broadcast(
                st[b]["wb_sb"][:, ich * CH : (ich + 1) * CH], st[b]["wr"][ich]))

        # ---- blend + store --------------------------------------------
        def mul(b, h, c0, c1):
            M, wb = st[b]["M"], st[b]["wb_sb"]
            seq("dve", nc.vector.tensor_mul(
                out=M[h][:, c0:c1], in0=M[h][:, c0:c1], in1=wb[:, c0:c1]))

        def add(b, h, c0, c1):
            A, Dv, M = st[b]["A"], st[b]["Dv"], st[b]["M"]
            seq("dve", nc.vector.tensor_add(
                out=A[h][:, c0:c1], in0=M[h][:, c0:c1], in1=Dv[h][:, c0:c1]))

        def store(b, h, c0, c1):
            seq("gp", nc.gpsimd.dma_start(
                out=out[b, h][:, c0:c1], in_=st[b]["A"][h][:, c0:c1]))

        # ---- emission schedule ----------------------------------------
        # scalar: warm the sigmoid table set during the idle load phase
        seq("act", nc.scalar.activation(out=sig_warm, in_=one_one,
                                        func=ACT.Sigmoid, bias=0.0, scale=1.0))
        for i in range(4):
            red(0, i)
        sub(0, 0)
        sub(0, 1)
        sub(0, 2)
        sub(0, 3)
        derive(0)
        head_pe(0)
        head_c16(0)
        head_v(0)

        red(1, 0)
        chunk_pe(0, 0); chunk_sig(0, 0); chunk_bc(0, 0)
        chunk_pe(0, 1); chunk_sig(0, 1); chunk_bc(0, 1)
        red(1, 1)
        sub(1, 0)
        sub(1, 1)
        for ich in range(2, 8):
            chunk_pe(0, ich); chunk_sig(0, ich); chunk_bc(0, ich)
        # blend0 col-block 0 while b1 tail loads land
        mul(0, 0, 0, LB); add(0, 0, 0, LB)
        mul(0, 1, 0, LB); add(0, 1, 0, LB)
        store(0, 0, 0, LB); store(0, 1, 0, LB)
        red(1, 2)
        red(1, 3)
        red(1, 4)
        sub(1, 2)
        sub(1, 3)
        sub(1, 4)
        derive(1)
        head_pe(1)
        head_c16(1)
        head_v(1)
        # blend0 col-block 1 on DVE while PE runs b1 values
        mul(0, 0, LB, N); add(0, 0, LB, N)
        mul(0, 1, LB, N); add(0, 1, LB, N)
        for ich in range(3):
            chunk_pe(1, ich); chunk_sig(1, ich); chunk_bc(1, ich)
        # b0 col-block-1 stores fill the DMA gap while b1 chunks trickle
        store(0, 0, LB, N); store(0, 1, LB, N)
        for ich in range(3, 8):
            chunk_pe(1, ich); chunk_sig(1, ich); chunk_bc(1, ich)
        # blend1 in 1024-col pieces trickling into stores
        for p in range(4):
            c0, c1 = p * PB, (p + 1) * PB
            mul(1, 0, c0, c1); add(1, 0, c0, c1)
            mul(1, 1, c0, c1); add(1, 1, c0, c1)
            store(1, 0, c0, c1); store(1, 1, c0, c1)


_NC_CACHE = None


def _get_nc():
    global _NC_CACHE
    if _NC_CACHE is None:
        _NC_CACHE = build_nc()
    return _NC_CACHE


def _make_in_maps(inputs):
    rgb = np.ascontiguousarray(np.asarray(inputs["rgb"], dtype=np.float32)).reshape(
        BS, PH, 128, N
    )
    evt = np.ascontiguousarray(np.asarray(inputs["evt"], dtype=np.float32)).reshape(
        BS, PH, 128, N
    )
    wpack = np.stack(
        [
            np.asarray(inputs[nm], dtype=np.float32).reshape(PH, 128, DIM).transpose(1, 0, 2)
            for nm in ("Wq_a", "Wk_a", "Wq_d", "Wk_d")
        ],
        axis=1,
    ).reshape(128, 4 * PH * DIM)
    bpack = np.stack(
        [
            np.asarray(inputs[nm], dtype=np.float32).reshape(PH, 128).T
            for nm in ("bq_a", "bk_a", "bq_d", "bk_d")
        ],
        axis=1,
    ).reshape(128, 4 * PH)
    base = {"wpack": np.ascontiguousarray(wpack), "bpack": np.ascontiguousarray(bpack)}
    in_maps = []
    for c in range(NCORES):
        m = dict(base)
        m["rgb"] = np.ascontiguousarray(rgb[c * BPC : (c + 1) * BPC])
        m["evt"] = np.ascontiguousarray(evt[c * BPC : (c + 1) * BPC])
        in_maps.append(m)
    return in_maps


def run(inputs, trace=False):
    nc = _get_nc()
    in_maps = _make_in_maps(inputs)
    res = run_bass_kernel_spmd(nc, in_maps, core_ids=list(range(NCORES)), trace=trace)
    outs = [
        np.asarray(res.results[i]["out"]).reshape(BPC, DIM, HH, WW)
        for i in range(NCORES)
    ]
    full = np.concatenate(outs, axis=0)
    return full, res


def kernel(**inputs) -> np.ndarray:
    full, _ = run(inputs, trace=False)
    return full


# revision 8
# speedup vs baseline: 2.1688x; 1.1339x over previous
"""Trainium2 Bass kernel for the two-branch sparse-attention fusion module.

Math (per batch b, tokens T = rgb/evt as (d=256, N=4096) d-major):
    s      = sum_n T[:, n]                           (256,)
    value[n] = T[:,n].v + c, v = (Wq^T Wk) s + N Wq^T bk, c = (Wk^T bq).s + N bq.bk
    w      = sigmoid((value_rgb - value_evt)/sqrt(d))
    out    = evt + w * (rgb - evt)

Dataflow (fp16 compute, fp32 DRAM I/O + PSUM accumulation):
    SP HWDGE   : packed weight/bias load (f32, 2 contiguous DMAs)
    gpsimd     : casting token loads f32->fp16, partition_broadcast of the
                 sigmoid row to 128 partitions, casting stores fp16->f32
    ScalarE    : streaming row-sum partials of A, sigmoid (psv->fp16 row)
    PE         : weight-product precompute, per-batch head matvecs, fp16
                 value matmuls (no hi/lo split; rel err ~1.3e-3 << 2e-2)
    DVE        : M = A-D with rowsum accum, blend M*=wb, A=M+D (fp16 2x)

Chunk-granular software pipeline. Each engine's program order is pinned
with explicit dependency chains (seq) in measured data-arrival order so
the tile scheduler cannot head-of-line block a late-data op ahead of
ready work. The last batch-1 blocks are split finer to shorten the tail.
Sharded data-parallel over batch: 8 cores x 2 batches, weights replicated.
"""

import numpy as np
from contextlib import ExitStack

import concourse.bass as bass
import concourse.tile as tile
from concourse import bacc, mybir
from concourse.bass import _add_dep_helper
from concourse.bass_utils import run_bass_kernel_spmd

F32 = mybir.dt.float32
FP16 = mybir.dt.float16

BS, DIM, HH, WW = 16, 256, 64, 64
N = HH * WW                 # 4096 tokens
NCORES = 8
BPC = BS // NCORES          # batches per core
PH = DIM // 128             # partition halves of the d dim
CH = 512                    # value-chunk (one PSUM bank of f32)
NCH = N // CH               # 8
LB = 2048                   # load block columns (1 MiB DRAM-side)
NLB = N // LB               # 2
PB = 1024                   # tail blend piece columns
INV_SQRT_D = 1.0 / 16.0


def build_nc() -> bass.Bass:
    nc = bacc.Bacc()

    rgb = nc.declare_dram_parameter("rgb", [BPC, PH, 128, N], F32, isOutput=False)
    evt = nc.declare_dram_parameter("evt", [BPC, PH, 128, N], F32, isOutput=False)
    # host-side packed weights/biases: one contiguous f32 row per partition
    # so the SP HWDGE load is a single clean 128-descriptor DMA each
    wpack = nc.declare_dram_parameter("wpack", [128, 4 * PH * DIM], F32, isOutput=False)
    bpack = nc.declare_dram_parameter("bpack", [128, 4 * PH], F32, isOutput=False)
    out = nc.declare_dram_parameter("out", [BPC, PH, 128, N], F32, isOutput=True)

    with tile.TileContext(nc) as tc:
        _body(tc, rgb, evt, wpack, bpack, out)
    nc.finalize()
    return nc


def _body(tc, rgb, evt, wpack, bpack, out):
    nc = tc.nc
    ACT = mybir.ActivationFunctionType

    chains = {}

    def seq(key, inst):
        # pin program order on one engine: inst runs after the previous
        # chained inst, making emission order authoritative
        prev = chains.get(key)
        if prev is not None:
            _add_dep_helper(inst.ins, prev.ins, sync=False, reason=f"order-{key}")
        chains[key] = inst
        return inst

    with ExitStack() as ctx:
        consts = ctx.enter_context(tc.tile_pool(name="consts", bufs=1))
        data = ctx.enter_context(tc.tile_pool(name="data", bufs=2))
        mpool = ctx.enter_context(tc.tile_pool(name="mpool", bufs=2))
        wbp = ctx.enter_context(tc.tile_pool(name="wbp", bufs=2))
        small = ctx.enter_context(tc.tile_pool(name="small", bufs=2))
        wrp = ctx.enter_context(tc.tile_pool(name="wrp", bufs=4))
        ps_val = ctx.enter_context(tc.tile_pool(name="ps_val", bufs=3, space="PSUM"))
        ps_wb = ctx.enter_context(tc.tile_pool(name="ps_wb", bufs=2, space="PSUM"))
        ps_pre = ctx.enter_context(tc.tile_pool(name="ps_pre", bufs=2, space="PSUM"))
        ps_head = ctx.enter_context(tc.tile_pool(name="ps_head", bufs=1, space="PSUM"))

        one_one = consts.tile([1, 1], FP16, tag="one_one")
        seq("dve", nc.vector.memset(one_one, 1.0))
        ones_row = consts.tile([1, 128], FP16, tag="ones_row")
        seq("dve", nc.vector.memset(ones_row, 1.0))
        garbage = consts.tile([128, 1], F32, tag="garbage")
        sig_warm = consts.tile([1, 1], F32, tag="sig_warm")

        # ---- weight loads (SP HWDGE, contiguous) ----------------------
        W, B = {}, {}
        wt = consts.tile([128, 4 * PH * DIM], F32, tag="wpack")
        nc.sync.dma_start(out=wt, in_=wpack[:, :])
        bt = consts.tile([128, 4 * PH], F32, tag="bpack")
        nc.sync.dma_start(out=bt, in_=bpack[:, :])
        for wi, nm in enumerate(("Wq_a", "Wk_a", "Wq_d", "Wk_d")):
            for h in range(PH):
                base = (wi * PH + h) * DIM
                W[(nm, h)] = wt[:, base : base + DIM]
        for bi, nm in enumerate(("bq_a", "bk_a", "bq_d", "bk_d")):
            for h in range(PH):
                B[(nm, h)] = bt[:, bi * PH + h : bi * PH + h + 1]

        # ---- token loads ----------------------------------------------
        # b0: 8 x 1MiB blocks. b1: same but the final (h1, blk1) pair is
        # split into 1024-col halves so the tail sub/red start sooner.
        st = [dict() for _ in range(BPC)]
        for b in range(BPC):
            A, Dv = {}, {}
            for h in range(PH):
                A[h] = data.tile([128, N], FP16, tag=f"A{h}", name=f"A{h}_{b}")
                Dv[h] = data.tile([128, N], FP16, tag=f"D{h}", name=f"D{h}_{b}")
            st[b].update(A=A, Dv=Dv)

        def emit_load(b, h, c0, c1):
            sl = slice(c0, c1)
            seq("gp", nc.gpsimd.dma_start(out=st[b]["A"][h][:, sl], in_=rgb[b, h][:, sl]))
            seq("gp", nc.gpsimd.dma_start(out=st[b]["Dv"][h][:, sl], in_=evt[b, h][:, sl]))

        for blk in range(NLB):
            for h in range(PH):
                emit_load(0, h, blk * LB, (blk + 1) * LB)
        emit_load(1, 0, 0, LB)
        emit_load(1, 1, 0, LB)
        emit_load(1, 0, LB, 2 * LB)
        emit_load(1, 1, LB, LB + PB)
        emit_load(1, 1, LB + PB, 2 * LB)

        # ---- precompute pieces (PE on f32 weights; DVE casts) ---------
        # emitted piecewise from the schedule so early b0 subs are not
        # head-of-line blocked behind cast ops in the DVE chain
        PT, U, R = {}, {}, {}

        def pre_pt(br, wq, wk, sign, jh):
            ps = ps_pre.tile([128, DIM], F32, tag="ps_pre", name=f"psPT{br}{jh}")
            for oh in range(PH):
                seq("pe", nc.tensor.matmul(
                    ps,
                    lhsT=W[(wk, oh)][:, jh * 128 : (jh + 1) * 128],
                    rhs=W[(wq, oh)],
                    start=(oh == 0), stop=(oh == PH - 1),
                ))
            t = consts.tile([128, DIM], FP16, tag=f"PT{br}{jh}", name=f"PT{br}{jh}")
            seq("dve", nc.vector.tensor_scalar_mul(out=t, in0=ps, scalar1=sign))
            PT[(br, jh)] = t

        def pre_ur_mm(br, wq, wk):
            ps = ps_pre.tile([128, 2 * PH], F32, tag="ps_pre", name=f"psUR{br}")
            for ih in range(PH):
                for oh in range(PH):
                    seq("pe", nc.tensor.matmul(
                        ps[:, ih : ih + 1],
                        lhsT=W[(wq, oh)][:, ih * 128 : (ih + 1) * 128],
                        rhs=B[("bk_" + br, oh)],
                        start=(oh == 0), stop=(oh == PH - 1),
                    ))
            for jh in range(PH):
                for oh in range(PH):
                    seq("pe", nc.tensor.matmul(
                        ps[:, PH + jh : PH + jh + 1],
                        lhsT=W[(wk, oh)][:, jh * 128 : (jh + 1) * 128],
                        rhs=B[("bq_" + br, oh)],
                        start=(oh == 0), stop=(oh == PH - 1),
                    ))
            return ps

        def pre_ur_cast(br, sign, ps):
            tU = consts.tile([128, PH], F32, tag=f"U{br}", name=f"U{br}")
            seq("dve", nc.vector.tensor_scalar_mul(out=tU, in0=ps[:, 0:PH], scalar1=float(sign * N)))
            tR = consts.tile([128, PH], FP16, tag=f"R{br}", name=f"R{br}")
            seq("dve", nc.vector.tensor_scalar_mul(out=tR, in0=ps[:, PH : 2 * PH], scalar1=sign))
            U[br], R[br] = tU, tR

        # ---- stage 1 ops ----------------------------------------------
        # partial-sum slots per batch: list of (h, c0, c1)
        SLOTS = [
            [(0, 0, LB), (1, 0, LB), (0, LB, 2 * LB), (1, LB, 2 * LB)],
            [(0, 0, LB), (1, 0, LB), (0, LB, 2 * LB), (1, LB, LB + PB), (1, LB + PB, 2 * LB)],
        ]
        for b in range(BPC):
            ns = len(SLOTS[b])
            st[b]["sa4"] = small.tile([128, ns], F32, tag="sa4", name=f"sa4_{b}")
            st[b]["sm4"] = small.tile([128, ns], F32, tag="sm4", name=f"sm4_{b}")
            st[b]["sa16"] = small.tile([128, ns], FP16, tag="sa16", name=f"sa16_{b}")
            st[b]["sd16"] = small.tile([128, ns], FP16, tag="sd16", name=f"sd16_{b}")
            M = {}
            for h in range(PH):
                M[h] = mpool.tile([128, N], FP16, tag=f"M{h}", name=f"M{h}_{b}")
            st[b]["M"] = M

        def red(b, i):
            h, c0, c1 = SLOTS[b][i]
            return seq("act", nc.scalar.activation(
                out=garbage.broadcast_to([128, c1 - c0]),
                in_=st[b]["A"][h][:, c0:c1],
                func=ACT.Copy,
                accum_out=st[b]["sa4"][:, i : i + 1],
            ))

        def sub(b, i):
            h, c0, c1 = SLOTS[b][i]
            return seq("dve", nc.vector.scalar_tensor_tensor(
                out=st[b]["M"][h][:, c0:c1],
                in0=st[b]["A"][h][:, c0:c1],
                scalar=1.0,
                in1=st[b]["Dv"][h][:, c0:c1],
                op0=mybir.AluOpType.mult,
                op1=mybir.AluOpType.subtract,
                accum_out=st[b]["sm4"][:, i : i + 1],
            ))

        def derive(b):
            with nc.allow_low_precision(reason="tiny fp16 partials"):
                seq("dve", nc.vector.tensor_scalar_mul(
                    out=st[b]["sa16"], in0=st[b]["sa4"], scalar1=1.0))
                seq("dve", nc.vector.tensor_sub(
                    out=st[b]["sd16"], in0=st[b]["sa4"], in1=st[b]["sm4"]))

        # batch-independent bias-dot part of c_diff: N*(bq_a.bk_a - bq_d.bk_d)
        c_bias = consts.tile([1, 1], FP16, tag="c_bias")

        def pre_cbias():
            ps = ps_pre.tile([1, 1], F32, tag="ps_pre", name="psCb")
            k = 0
            for bq, bk, sgn in (("bq_a", "bk_a", 1), ("bq_d", "bk_d", -1)):
                for oh in range(PH):
                    t = consts.tile([128, 1], F32, tag=f"bkN{bk}{oh}", name=f"bkN{bk}{oh}")
                    seq("dve", nc.vector.tensor_scalar_mul(
                        out=t, in0=B[(bk, oh)], scalar1=float(sgn * N)))
                    seq("pe", nc.tensor.matmul(ps, lhsT=B[(bq, oh)], rhs=t,
                                               start=(k == 0), stop=(k == 3)))
                    k += 1
            seq("dve", nc.vector.tensor_scalar_mul(out=c_bias, in0=ps, scalar1=1.0))

        # ---- head -----------------------------------------------------
        def head_pe(b):
            sa16, sd16 = st[b]["sa16"], st[b]["sd16"]
            S4 = {"a": sa16, "d": sd16}
            ps_c = ps_head.tile([1, 1], F32, tag="ps_h", name=f"psc_{b}")
            terms = [
                (S4[br][:, i : i + 1], R[br][:, SLOTS[b][i][0] : SLOTS[b][i][0] + 1])
                for br in ("a", "d")
                for i in range(len(SLOTS[b]))
            ]
            for i, (l, r) in enumerate(terms):
                seq("pe", nc.tensor.matmul(ps_c, lhsT=l, rhs=r, start=(i == 0), stop=False))
            seq("pe", nc.tensor.matmul(ps_c, lhsT=c_bias, rhs=one_one, start=False, stop=True))
            psv = {}
            for br in ("a", "d"):
                ps = ps_head.tile([128, PH], F32, tag="ps_h", name=f"psv{br}_{b}")
                for ih in range(PH):
                    nslots = len(SLOTS[b])
                    for i in range(nslots):
                        jh = SLOTS[b][i][0]
                        seq("pe", nc.tensor.matmul(
                            ps[:, ih : ih + 1],
                            lhsT=PT[(br, jh)][:, ih * 128 : (ih + 1) * 128],
                            rhs=S4[br][:, i : i + 1],
                            start=(i == 0), stop=(i == nslots - 1),
                        ))
                psv[br] = ps
            st[b]["ps_c"], st[b]["ps_v"] = ps_c, psv

        def head_c16(b):
            c16 = small.tile([1, 1], F32, tag="c16", name=f"c16_{b}")
            seq("act", nc.scalar.mul(out=c16, in_=st[b]["ps_c"], mul=INV_SQRT_D))
            st[b]["c16"] = c16

        def head_v(b):
            VH = {}
            for br in ("a", "d"):
                v = small.tile([128, PH], F32, tag=f"v{br}", name=f"v{br}_{b}")
                seq("dve", nc.vector.tensor_add(out=v, in0=st[b]["ps_v"][br], in1=U[br]))
                vh = small.tile([128, PH], FP16, tag=f"vh{br}", name=f"vh{br}_{b}")
                with nc.allow_low_precision(reason="fp16 matvec vector"):
                    seq("dve", nc.vector.tensor_scalar_mul(out=vh, in0=v, scalar1=1.0))
                VH[br] = vh
            st[b]["VH"] = VH

        # ---- stage 2: per 512-chunk value -> sigmoid -> broadcast -----
        for b in range(BPC):
            st[b]["wb_sb"] = wbp.tile([128, N], FP16, tag="wb_sb", name=f"wb_sb_{b}")
            st[b]["wr"] = {}

        def chunk_pe(b, ich):
            VH, A, Dv = st[b]["VH"], st[b]["A"], st[b]["Dv"]
            sl = slice(ich * CH, (ich + 1) * CH)
            psv = ps_val.tile([1, CH], F32, tag="psv", name=f"psval{ich}_{b}")
            mms = [
                (VH["a"][:, 0:1], A[0]), (VH["a"][:, 1:2], A[1]),
                (VH["d"][:, 0:1], Dv[0]), (VH["d"][:, 1:2], Dv[1]),
            ]
            for i, (v, t) in enumerate(mms):
                seq("pe", nc.tensor.matmul(psv, lhsT=v, rhs=t[:, sl],
                                           start=(i == 0), stop=(i == len(mms) - 1)))
            st[b]["wr"][ich] = psv

        def chunk_sig(b, ich):
            wrow = wrp.tile([1, CH], FP16, tag="wr", name=f"wrow{ich}_{b}")
            seq("act", nc.scalar.activation(
                out=wrow, in_=st[b]["wr"][ich],
                func=ACT.Sigmoid, bias=st[b]["c16"], scale=INV_SQRT_D,
            ))
            st[b]["wr"][ich] = wrow

        def chunk_bcmm(b, ich):
            # K=1 broadcast matmul: wb[128,512] = ones^T @ wrow
            wb = ps_wb.tile([128, CH], F32, tag="wb", name=f"wb{ich}_{b}")
            seq("pe", nc.tensor.matmul(wb, lhsT=ones_row, rhs=st[b]["wr"][ich],
                                       start=True, stop=True))
            st[b]["wbps"] = st[b].get("wbps", {})
            st[b]["wbps"][ich] = wb

        def chunk_copy(b, ich):
            seq("act", nc.scalar.copy(
                out=st[b]["wb_sb"][:, ich * CH : (ich + 1) * CH],
                in_=st[b]["wbps"][ich]))

        # ---- blend + store --------------------------------------------
        def mul(b, h, c0, c1):
            M, wb = st[b]["M"], st[b]["wb_sb"]
            seq("dve", nc.vector.tensor_mul(
                out=M[h][:, c0:c1], in0=M[h][:, c0:c1], in1=wb[:, c0:c1]))

        def add(b, h, c0, c1):
            A, Dv, M = st[b]["A"], st[b]["Dv"], st[b]["M"]
            seq("dve", nc.vector.tensor_add(
                out=A[h][:, c0:c1], in0=M[h][:, c0:c1], in1=Dv[h][:, c0:c1]))

        def store(b, h, c0, c1):
            seq("gp", nc.gpsimd.dma_start(
                out=out[b, h][:, c0:c1], in_=st[b]["A"][h][:, c0:c1]))

        # ---- emission schedule ----------------------------------------
        # scalar: warm the sigmoid table set during the idle load phase
        seq("act", nc.scalar.activation(out=sig_warm, in_=one_one,
                                        func=ACT.Sigmoid, bias=0.0, scale=1.0))
        for i in range(4):
            red(0, i)
        # precompute phases interleaved with b0 subs in data-arrival order
        pre_pt("a", "Wq_a", "Wk_a", 1.0, 0)
        pre_pt("a", "Wq_a", "Wk_a", 1.0, 1)
        sub(0, 0)
        ps_ura = pre_ur_mm("a", "Wq_a", "Wk_a")
        pre_ur_cast("a", 1.0, ps_ura)
        sub(0, 1)
        pre_pt("d", "Wq_d", "Wk_d", -1.0, 0)
        pre_pt("d", "Wq_d", "Wk_d", -1.0, 1)
        sub(0, 2)
        ps_urd = pre_ur_mm("d", "Wq_d", "Wk_d")
        pre_ur_cast("d", -1.0, ps_urd)
        pre_cbias()
        sub(0, 3)
        derive(0)
        head_pe(0)
        head_c16(0)
        head_v(0)

        red(1, 0)
        chunk_pe(0, 0); chunk_sig(0, 0)
        chunk_pe(0, 1); chunk_bcmm(0, 0); chunk_sig(0, 1); chunk_copy(0, 0)
        red(1, 1)
        sub(1, 0)
        sub(1, 1)
        for ich in range(2, 8):
            chunk_pe(0, ich); chunk_bcmm(0, ich - 1)
            chunk_sig(0, ich); chunk_copy(0, ich - 1)
        chunk_bcmm(0, 7); chunk_copy(0, 7)
        # blend0 col-block 0 while b1 tail loads land
        mul(0, 0, 0, LB); add(0, 0, 0, LB)
        mul(0, 1, 0, LB); add(0, 1, 0, LB)
        store(0, 0, 0, LB); store(0, 1, 0, LB)
        red(1, 2)
        red(1, 3)
        red(1, 4)
        sub(1, 2)
        sub(1, 3)
        sub(1, 4)
        derive(1)
        head_pe(1)
        head_c16(1)
        head_v(1)
        # blend0 col-block 1 on DVE while PE runs b1 values
        mul(0, 0, LB, N); add(0, 0, LB, N)
        mul(0, 1, LB, N); add(0, 1, LB, N)
        chunk_pe(1, 0); chunk_sig(1, 0)
        chunk_pe(1, 1); chunk_bcmm(1, 0); chunk_sig(1, 1); chunk_copy(1, 0)
        chunk_pe(1, 2); chunk_bcmm(1, 1); chunk_sig(1, 2); chunk_copy(1, 1)
        # b0 col-block-1 stores fill the DMA gap while b1 chunks trickle
        store(0, 0, LB, N); store(0, 1, LB, N)
        for ich in range(3, 8):
            chunk_pe(1, ich); chunk_bcmm(1, ich - 1)
            chunk_sig(1, ich); chunk_copy(1, ich - 1)
        chunk_bcmm(1, 7); chunk_copy(1, 7)
        # blend1 in 1024-col pieces trickling into stores
        for p in range(4):
            c0, c1 = p * PB, (p + 1) * PB
            mul(1, 0, c0, c1); add(1, 0, c0, c1)
            mul(1, 1, c0, c1); add(1, 1, c0, c1)
            store(1, 0, c0, c1); store(1, 1, c0, c1)


_NC_CACHE = None


def _get_nc():
    global _NC_CACHE
    if _NC_CACHE is None:
        _NC_CACHE = build_nc()
    return _NC_CACHE


def _make_in_maps(inputs):
    rgb = np.ascontiguousarray(np.asarray(inputs["rgb"], dtype=np.float32)).reshape(
        BS, PH, 128, N
    )
    evt = np.ascontiguousarray(np.asarray(inputs["evt"], dtype=np.float32)).reshape(
        BS, PH, 128, N
    )
    wpack = np.stack(
        [
            np.asarray(inputs[nm], dtype=np.float32).reshape(PH, 128, DIM).transpose(1, 0, 2)
            for nm in ("Wq_a", "Wk_a", "Wq_d", "Wk_d")
        ],
        axis=1,
    ).reshape(128, 4 * PH * DIM)
    bpack = np.stack(
        [
            np.asarray(inputs[nm], dtype=np.float32).reshape(PH, 128).T
            for nm in ("bq_a", "bk_a", "bq_d", "bk_d")
        ],
        axis=1,
    ).reshape(128, 4 * PH)
    base = {"wpack": np.ascontiguousarray(wpack), "bpack": np.ascontiguousarray(bpack)}
    in_maps = []
    for c in range(NCORES):
        m = dict(base)
        m["rgb"] = np.ascontiguousarray(rgb[c * BPC : (c + 1) * BPC])
        m["evt"] = np.ascontiguousarray(evt[c * BPC : (c + 1) * BPC])
        in_maps.append(m)
    return in_maps


def run(inputs, trace=False):
    nc = _get_nc()
    in_maps = _make_in_maps(inputs)
    res = run_bass_kernel_spmd(nc, in_maps, core_ids=list(range(NCORES)), trace=trace)
    outs = [
        np.asarray(res.results[i]["out"]).reshape(BPC, DIM, HH, WW)
        for i in range(NCORES)
    ]
    full = np.concatenate(outs, axis=0)
    return full, res


def kernel(**inputs) -> np.ndarray:
    full, _ = run(inputs, trace=False)
    return full


# revision 9
# speedup vs baseline: 2.3952x; 1.1044x over previous
"""Trainium2 Bass kernel for the two-branch sparse-attention fusion module.

Math (per batch b, tokens T = rgb/evt as (d=256, N=4096) d-major):
    s      = sum_n T[:, n]                           (256,)
    value[n] = T[:,n].v + c, v = (Wq^T Wk) s + N Wq^T bk, c = (Wk^T bq).s + N bq.bk
    w      = sigmoid((value_rgb - value_evt)/sqrt(d))
    out    = evt + w * (rgb - evt)

Dataflow (fp16 compute, fp32 DRAM I/O + PSUM accumulation):
    SP HWDGE   : packed weight/bias load (f32, 2 contiguous DMAs)
    gpsimd     : casting token loads f32->fp16, partition_broadcast of the
                 sigmoid row to 128 partitions, casting stores fp16->f32
    ScalarE    : streaming row-sum partials of A, sigmoid (psv->fp16 row)
    PE         : weight-product precompute, per-batch head matvecs, fp16
                 value matmuls (no hi/lo split; rel err ~1.3e-3 << 2e-2)
    DVE        : M = A-D with rowsum accum, blend M*=wb, A=M+D (fp16 2x)

Chunk-granular software pipeline. Each engine's program order is pinned
with explicit dependency chains (seq) in measured data-arrival order so
the tile scheduler cannot head-of-line block a late-data op ahead of
ready work. The last batch-1 blocks are split finer to shorten the tail.
Sharded data-parallel over batch: 8 cores x 2 batches, weights replicated.
"""

import numpy as np
from contextlib import ExitStack

import concourse.bass as bass
import concourse.tile as tile
from concourse import bacc, mybir
from concourse.bass import _add_dep_helper
from concourse.bass_utils import run_bass_kernel_spmd

F32 = mybir.dt.float32
FP16 = mybir.dt.float16

BS, DIM, HH, WW = 16, 256, 64, 64
N = HH * WW                 # 4096 tokens
NCORES = 8
BPC = BS // NCORES          # batches per core
PH = DIM // 128             # partition halves of the d dim
CH = 512                    # value-chunk (one PSUM bank of f32)
NCH = N // CH               # 8
LB = 2048                   # load block columns (1 MiB DRAM-side)
NLB = N // LB               # 2
PB = 1024                   # tail blend piece columns
INV_SQRT_D = 1.0 / 16.0


def build_nc() -> bass.Bass:
    nc = bacc.Bacc()

    rgb = nc.declare_dram_parameter("rgb", [BPC, PH, 128, N], F32, isOutput=False)
    evt = nc.declare_dram_parameter("evt", [BPC, PH, 128, N], F32, isOutput=False)
    # host-side packed weights/biases: one contiguous f32 row per partition
    # so the SP HWDGE load is a single clean 128-descriptor DMA each
    wpack = nc.declare_dram_parameter("wpack", [128, 4 * PH * DIM], F32, isOutput=False)
    bpack = nc.declare_dram_parameter("bpack", [128, 4 * PH], F32, isOutput=False)
    out = nc.declare_dram_parameter("out", [BPC, PH, 128, N], F32, isOutput=True)

    with tile.TileContext(nc) as tc:
        _body(tc, rgb, evt, wpack, bpack, out)
    nc.finalize()
    return nc


def _body(tc, rgb, evt, wpack, bpack, out):
    nc = tc.nc
    ACT = mybir.ActivationFunctionType

    chains = {}

    def seq(key, inst):
        # pin program order on one engine: inst runs after the previous
        # chained inst, making emission order authoritative
        prev = chains.get(key)
        if prev is not None:
            _add_dep_helper(inst.ins, prev.ins, sync=False, reason=f"order-{key}")
        chains[key] = inst
        return inst

    with ExitStack() as ctx:
        consts = ctx.enter_context(tc.tile_pool(name="consts", bufs=1))
        data = ctx.enter_context(tc.tile_pool(name="data", bufs=2))
        mpool = ctx.enter_context(tc.tile_pool(name="mpool", bufs=2))
        wbp = ctx.enter_context(tc.tile_pool(name="wbp", bufs=2))
        small = ctx.enter_context(tc.tile_pool(name="small", bufs=2))
        wrp = ctx.enter_context(tc.tile_pool(name="wrp", bufs=4))
        ps_val = ctx.enter_context(tc.tile_pool(name="ps_val", bufs=3, space="PSUM"))
        ps_wb = ctx.enter_context(tc.tile_pool(name="ps_wb", bufs=2, space="PSUM"))
        ps_pre = ctx.enter_context(tc.tile_pool(name="ps_pre", bufs=2, space="PSUM"))
        ps_head = ctx.enter_context(tc.tile_pool(name="ps_head", bufs=1, space="PSUM"))

        one_one = consts.tile([1, 1], FP16, tag="one_one")
        seq("dve", nc.vector.memset(one_one, 1.0))
        ones_row = consts.tile([1, 128], FP16, tag="ones_row")
        seq("dve", nc.vector.memset(ones_row, 1.0))
        garbage = consts.tile([128, 1], F32, tag="garbage")
        sig_warm = consts.tile([1, 1], F32, tag="sig_warm")

        # ---- weight loads (SP HWDGE, contiguous) ----------------------
        W, B = {}, {}
        wt = consts.tile([128, 4 * PH * DIM], F32, tag="wpack")
        nc.sync.dma_start(out=wt, in_=wpack[:, :])
        bt = consts.tile([128, 4 * PH], F32, tag="bpack")
        nc.sync.dma_start(out=bt, in_=bpack[:, :])
        for wi, nm in enumerate(("Wq_a", "Wk_a", "Wq_d", "Wk_d")):
            for h in range(PH):
                base = (wi * PH + h) * DIM
                W[(nm, h)] = wt[:, base : base + DIM]
        for bi, nm in enumerate(("bq_a", "bk_a", "bq_d", "bk_d")):
            for h in range(PH):
                B[(nm, h)] = bt[:, bi * PH + h : bi * PH + h + 1]

        # ---- token loads ----------------------------------------------
        # b0: 8 x 1MiB blocks. b1: same but the final (h1, blk1) pair is
        # split into 1024-col halves so the tail sub/red start sooner.
        st = [dict() for _ in range(BPC)]
        for b in range(BPC):
            A, Dv = {}, {}
            for h in range(PH):
                A[h] = data.tile([128, N], FP16, tag=f"A{h}", name=f"A{h}_{b}")
                Dv[h] = data.tile([128, N], FP16, tag=f"D{h}", name=f"D{h}_{b}")
            st[b].update(A=A, Dv=Dv)

        def emit_load(b, h, c0, c1):
            sl = slice(c0, c1)
            seq("gp", nc.gpsimd.dma_start(out=st[b]["A"][h][:, sl], in_=rgb[b, h][:, sl]))
            seq("gp", nc.gpsimd.dma_start(out=st[b]["Dv"][h][:, sl], in_=evt[b, h][:, sl]))

        for blk in range(NLB):
            for h in range(PH):
                emit_load(0, h, blk * LB, (blk + 1) * LB)
        emit_load(1, 0, 0, LB)
        emit_load(1, 1, 0, LB)
        emit_load(1, 0, LB, 2 * LB)
        emit_load(1, 1, LB, LB + PB)
        emit_load(1, 1, LB + PB, LB + PB + 512)
        emit_load(1, 1, LB + PB + 512, 2 * LB)

        # ---- precompute pieces (PE on f32 weights; DVE casts) ---------
        # emitted piecewise from the schedule so early b0 subs are not
        # head-of-line blocked behind cast ops in the DVE chain
        PT, U, R = {}, {}, {}

        def pre_pt(br, wq, wk, sign, jh):
            ps = ps_pre.tile([128, DIM], F32, tag="ps_pre", name=f"psPT{br}{jh}")
            for oh in range(PH):
                seq("pe", nc.tensor.matmul(
                    ps,
                    lhsT=W[(wk, oh)][:, jh * 128 : (jh + 1) * 128],
                    rhs=W[(wq, oh)],
                    start=(oh == 0), stop=(oh == PH - 1),
                ))
            t = consts.tile([128, DIM], FP16, tag=f"PT{br}{jh}", name=f"PT{br}{jh}")
            seq("act", nc.scalar.mul(out=t, in_=ps, mul=sign))
            PT[(br, jh)] = t

        def pre_ur_mm(br, wq, wk):
            ps = ps_pre.tile([128, 2 * PH], F32, tag="ps_pre", name=f"psUR{br}")
            for ih in range(PH):
                for oh in range(PH):
                    seq("pe", nc.tensor.matmul(
                        ps[:, ih : ih + 1],
                        lhsT=W[(wq, oh)][:, ih * 128 : (ih + 1) * 128],
                        rhs=B[("bk_" + br, oh)],
                        start=(oh == 0), stop=(oh == PH - 1),
                    ))
            for jh in range(PH):
                for oh in range(PH):
                    seq("pe", nc.tensor.matmul(
                        ps[:, PH + jh : PH + jh + 1],
                        lhsT=W[(wk, oh)][:, jh * 128 : (jh + 1) * 128],
                        rhs=B[("bq_" + br, oh)],
                        start=(oh == 0), stop=(oh == PH - 1),
                    ))
            return ps

        def pre_ur_cast(br, sign, ps):
            tU = consts.tile([128, PH], F32, tag=f"U{br}", name=f"U{br}")
            seq("act", nc.scalar.mul(out=tU, in_=ps[:, 0:PH], mul=float(sign * N)))
            tR = consts.tile([128, PH], FP16, tag=f"R{br}", name=f"R{br}")
            seq("act", nc.scalar.mul(out=tR, in_=ps[:, PH : 2 * PH], mul=sign))
            U[br], R[br] = tU, tR

        # ---- stage 1 ops ----------------------------------------------
        # partial-sum slots per batch: list of (h, c0, c1)
        SLOTS = [
            [(0, 0, LB), (1, 0, LB), (0, LB, 2 * LB), (1, LB, 2 * LB)],
            [(0, 0, LB), (1, 0, LB), (0, LB, 2 * LB), (1, LB, LB + PB),
             (1, LB + PB, LB + PB + 512), (1, LB + PB + 512, 2 * LB)],
        ]
        for b in range(BPC):
            ns = len(SLOTS[b])
            st[b]["sa4"] = small.tile([128, ns], F32, tag="sa4", name=f"sa4_{b}")
            st[b]["sm4"] = small.tile([128, ns], F32, tag="sm4", name=f"sm4_{b}")
            st[b]["sa16"] = small.tile([128, ns], FP16, tag="sa16", name=f"sa16_{b}")
            st[b]["sd16"] = small.tile([128, ns], FP16, tag="sd16", name=f"sd16_{b}")
            M = {}
            for h in range(PH):
                M[h] = mpool.tile([128, N], FP16, tag=f"M{h}", name=f"M{h}_{b}")
            st[b]["M"] = M

        def red(b, i):
            h, c0, c1 = SLOTS[b][i]
            return seq("act", nc.scalar.activation(
                out=garbage.broadcast_to([128, c1 - c0]),
                in_=st[b]["A"][h][:, c0:c1],
                func=ACT.Copy,
                accum_out=st[b]["sa4"][:, i : i + 1],
            ))

        def sub(b, i):
            h, c0, c1 = SLOTS[b][i]
            return seq("dve", nc.vector.scalar_tensor_tensor(
                out=st[b]["M"][h][:, c0:c1],
                in0=st[b]["A"][h][:, c0:c1],
                scalar=1.0,
                in1=st[b]["Dv"][h][:, c0:c1],
                op0=mybir.AluOpType.mult,
                op1=mybir.AluOpType.subtract,
                accum_out=st[b]["sm4"][:, i : i + 1],
            ))

        def derive(b):
            with nc.allow_low_precision(reason="tiny fp16 partials"):
                seq("dve", nc.vector.tensor_scalar_mul(
                    out=st[b]["sa16"], in0=st[b]["sa4"], scalar1=1.0))
                seq("dve", nc.vector.tensor_sub(
                    out=st[b]["sd16"], in0=st[b]["sa4"], in1=st[b]["sm4"]))

        # batch-independent bias-dot part of c_diff: N*(bq_a.bk_a - bq_d.bk_d)
        c_bias = consts.tile([1, 1], FP16, tag="c_bias")

        def pre_cbias():
            ps = ps_pre.tile([1, 1], F32, tag="ps_pre", name="psCb")
            k = 0
            for bq, bk, sgn in (("bq_a", "bk_a", 1), ("bq_d", "bk_d", -1)):
                for oh in range(PH):
                    t = consts.tile([128, 1], F32, tag=f"bkN{bk}{oh}", name=f"bkN{bk}{oh}")
                    seq("act", nc.scalar.mul(out=t, in_=B[(bk, oh)], mul=float(sgn * N)))
                    seq("pe", nc.tensor.matmul(ps, lhsT=B[(bq, oh)], rhs=t,
                                               start=(k == 0), stop=(k == 3)))
                    k += 1
            seq("act", nc.scalar.mul(out=c_bias, in_=ps, mul=1.0))

        # ---- head -----------------------------------------------------
        def head_pe(b):
            sa16, sd16 = st[b]["sa16"], st[b]["sd16"]
            S4 = {"a": sa16, "d": sd16}
            ps_c = ps_head.tile([1, 1], F32, tag="ps_h", name=f"psc_{b}")
            terms = [
                (S4[br][:, i : i + 1], R[br][:, SLOTS[b][i][0] : SLOTS[b][i][0] + 1])
                for br in ("a", "d")
                for i in range(len(SLOTS[b]))
            ]
            for i, (l, r) in enumerate(terms):
                seq("pe", nc.tensor.matmul(ps_c, lhsT=l, rhs=r, start=(i == 0), stop=False))
            seq("pe", nc.tensor.matmul(ps_c, lhsT=c_bias, rhs=one_one, start=False, stop=True))
            psv = {}
            for br in ("a", "d"):
                ps = ps_head.tile([128, PH], F32, tag="ps_h", name=f"psv{br}_{b}")
                for ih in range(PH):
                    nslots = len(SLOTS[b])
                    for i in range(nslots):
                        jh = SLOTS[b][i][0]
                        seq("pe", nc.tensor.matmul(
                            ps[:, ih : ih + 1],
                            lhsT=PT[(br, jh)][:, ih * 128 : (ih + 1) * 128],
                            rhs=S4[br][:, i : i + 1],
                            start=(i == 0), stop=(i == nslots - 1),
                        ))
                psv[br] = ps
            st[b]["ps_c"], st[b]["ps_v"] = ps_c, psv

        def head_c16(b):
            c16 = small.tile([1, 1], F32, tag="c16", name=f"c16_{b}")
            seq("act", nc.scalar.mul(out=c16, in_=st[b]["ps_c"], mul=INV_SQRT_D))
            st[b]["c16"] = c16

        def head_v(b):
            VH = {}
            for br in ("a", "d"):
                v = small.tile([128, PH], F32, tag=f"v{br}", name=f"v{br}_{b}")
                seq("dve", nc.vector.tensor_add(out=v, in0=st[b]["ps_v"][br], in1=U[br]))
                vh = small.tile([128, PH], FP16, tag=f"vh{br}", name=f"vh{br}_{b}")
                with nc.allow_low_precision(reason="fp16 matvec vector"):
                    seq("dve", nc.vector.tensor_scalar_mul(out=vh, in0=v, scalar1=1.0))
                VH[br] = vh
            st[b]["VH"] = VH

        # ---- stage 2: per 512-chunk value -> sigmoid -> broadcast -----
        for b in range(BPC):
            st[b]["wb_sb"] = wbp.tile([128, N], FP16, tag="wb_sb", name=f"wb_sb_{b}")
            st[b]["wr"] = {}

        def chunk_pe(b, ich):
            VH, A, Dv = st[b]["VH"], st[b]["A"], st[b]["Dv"]
            sl = slice(ich * CH, (ich + 1) * CH)
            psv = ps_val.tile([1, CH], F32, tag="psv", name=f"psval{ich}_{b}")
            mms = [
                (VH["a"][:, 0:1], A[0]), (VH["a"][:, 1:2], A[1]),
                (VH["d"][:, 0:1], Dv[0]), (VH["d"][:, 1:2], Dv[1]),
            ]
            for i, (v, t) in enumerate(mms):
                seq("pe", nc.tensor.matmul(psv, lhsT=v, rhs=t[:, sl],
                                           start=(i == 0), stop=(i == len(mms) - 1)))
            st[b]["wr"][ich] = psv

        def chunk_sig(b, ich):
            wrow = wrp.tile([1, CH], FP16, tag="wr", name=f"wrow{ich}_{b}")
            seq("act", nc.scalar.activation(
                out=wrow, in_=st[b]["wr"][ich],
                func=ACT.Sigmoid, bias=st[b]["c16"], scale=INV_SQRT_D,
            ))
            st[b]["wr"][ich] = wrow

        def chunk_bcmm(b, ich):
            # K=1 broadcast matmul: wb[128,512] = ones^T @ wrow
            wb = ps_wb.tile([128, CH], F32, tag="wb", name=f"wb{ich}_{b}")
            seq("pe", nc.tensor.matmul(wb, lhsT=ones_row, rhs=st[b]["wr"][ich],
                                       start=True, stop=True))
            st[b]["wbps"] = st[b].get("wbps", {})
            st[b]["wbps"][ich] = wb

        def chunk_copy(b, ich):
            seq("act", nc.scalar.copy(
                out=st[b]["wb_sb"][:, ich * CH : (ich + 1) * CH],
                in_=st[b]["wbps"][ich]))

        # ---- blend + store --------------------------------------------
        def mul(b, h, c0, c1):
            M, wb = st[b]["M"], st[b]["wb_sb"]
            seq("dve", nc.vector.tensor_mul(
                out=M[h][:, c0:c1], in0=M[h][:, c0:c1], in1=wb[:, c0:c1]))

        def add(b, h, c0, c1):
            A, Dv, M = st[b]["A"], st[b]["Dv"], st[b]["M"]
            seq("dve", nc.vector.tensor_add(
                out=A[h][:, c0:c1], in0=M[h][:, c0:c1], in1=Dv[h][:, c0:c1]))

        def store(b, h, c0, c1):
            seq("gp", nc.gpsimd.dma_start(
                out=out[b, h][:, c0:c1], in_=st[b]["A"][h][:, c0:c1]))

        # ---- emission schedule ----------------------------------------
        # scalar: warm the sigmoid table set during the idle load phase
        seq("act", nc.scalar.activation(out=sig_warm, in_=one_one,
                                        func=ACT.Sigmoid, bias=0.0, scale=1.0))
        # precompute (PE + scalar casts) interleaved with red0; subs on DVE
        pre_pt("a", "Wq_a", "Wk_a", 1.0, 0)
        red(0, 0)
        pre_pt("a", "Wq_a", "Wk_a", 1.0, 1)
        red(0, 1)
        ps_ura = pre_ur_mm("a", "Wq_a", "Wk_a")
        pre_ur_cast("a", 1.0, ps_ura)
        pre_pt("d", "Wq_d", "Wk_d", -1.0, 0)
        red(0, 2)
        pre_pt("d", "Wq_d", "Wk_d", -1.0, 1)
        ps_urd = pre_ur_mm("d", "Wq_d", "Wk_d")
        pre_ur_cast("d", -1.0, ps_urd)
        pre_cbias()
        red(0, 3)
        sub(0, 0)
        sub(0, 1)
        sub(0, 2)
        sub(0, 3)
        derive(0)
        head_pe(0)
        head_c16(0)
        head_v(0)

        red(1, 0)
        chunk_pe(0, 0); chunk_sig(0, 0)
        chunk_pe(0, 1); chunk_bcmm(0, 0); chunk_sig(0, 1); chunk_copy(0, 0)
        red(1, 1)
        sub(1, 0)
        sub(1, 1)
        for ich in range(2, 8):
            chunk_pe(0, ich); chunk_bcmm(0, ich - 1)
            chunk_sig(0, ich); chunk_copy(0, ich - 1)
        chunk_bcmm(0, 7); chunk_copy(0, 7)
        # blend0 col-block 0 while b1 tail loads land
        mul(0, 0, 0, LB); add(0, 0, 0, LB)
        mul(0, 1, 0, LB); add(0, 1, 0, LB)
        store(0, 0, 0, LB); store(0, 1, 0, LB)
        red(1, 2)
        red(1, 3)
        red(1, 4)
        red(1, 5)
        sub(1, 2)
        sub(1, 3)
        sub(1, 4)
        sub(1, 5)
        derive(1)
        head_pe(1)
        head_c16(1)
        head_v(1)
        # blend0 col-block 1 on DVE while PE runs b1 values
        mul(0, 0, LB, N); add(0, 0, LB, N)
        mul(0, 1, LB, N); add(0, 1, LB, N)
        chunk_pe(1, 0); chunk_sig(1, 0)
        chunk_pe(1, 1); chunk_bcmm(1, 0); chunk_sig(1, 1); chunk_copy(1, 0)
        chunk_pe(1, 2); chunk_bcmm(1, 1); chunk_sig(1, 2); chunk_copy(1, 1)
        # b0 col-block-1 stores fill the DMA gap while b1 chunks trickle
        store(0, 0, LB, N); store(0, 1, LB, N)
        for ich in range(3, 8):
            chunk_pe(1, ich); chunk_bcmm(1, ich - 1)
            chunk_sig(1, ich); chunk_copy(1, ich - 1)
        chunk_bcmm(1, 7); chunk_copy(1, 7)
        # blend1 in 1024-col pieces trickling into stores
        for p in range(4):
            c0, c1 = p * PB, (p + 1) * PB
            mul(1, 0, c0, c1); add(1, 0, c0, c1)
            mul(1, 1, c0, c1); add(1, 1, c0, c1)
            store(1, 0, c0, c1); store(1, 1, c0, c1)


_NC_CACHE = None


def _get_nc():
    global _NC_CACHE
    if _NC_CACHE is None:
        _NC_CACHE = build_nc()
    return _NC_CACHE


def _make_in_maps(inputs):
    rgb = np.ascontiguousarray(np.asarray(inputs["rgb"], dtype=np.float32)).reshape(
        BS, PH, 128, N
    )
    evt = np.ascontiguousarray(np.asarray(inputs["evt"], dtype=np.float32)).reshape(
        BS, PH, 128, N
    )
    wpack = np.stack(
        [
            np.asarray(inputs[nm], dtype=np.float32).reshape(PH, 128, DIM).transpose(1, 0, 2)
            for nm in ("Wq_a", "Wk_a", "Wq_d", "Wk_d")
        ],
        axis=1,
    ).reshape(128, 4 * PH * DIM)
    bpack = np.stack(
        [
            np.asarray(inputs[nm], dtype=np.float32).reshape(PH, 128).T
            for nm in ("bq_a", "bk_a", "bq_d", "bk_d")
        ],
        axis=1,
    ).reshape(128, 4 * PH)
    base = {"wpack": np.ascontiguousarray(wpack), "bpack": np.ascontiguousarray(bpack)}
    in_maps = []
    for c in range(NCORES):
        m = dict(base)
        m["rgb"] = np.ascontiguousarray(rgb[c * BPC : (c + 1) * BPC])
        m["evt"] = np.ascontiguousarray(evt[c * BPC : (c + 1) * BPC])
        in_maps.append(m)
    return in_maps


def run(inputs, trace=False):
    nc = _get_nc()
    in_maps = _make_in_maps(inputs)
    res = run_bass_kernel_spmd(nc, in_maps, core_ids=list(range(NCORES)), trace=trace)
    outs = [
        np.asarray(res.results[i]["out"]).reshape(BPC, DIM, HH, WW)
        for i in range(NCORES)
    ]
    full = np.concatenate(outs, axis=0)
    return full, res


def kernel(**inputs) -> np.ndarray:
    full, _ = run(inputs, trace=False)
    return full
